# revision 1
# baseline (speedup 1.0000x reference)
"""Trainium2 Bass kernel for nn_Node_Convolution (GNN message passing).

Reference computation:
    z   = concat([x[src], x[tgt], edge_attr], -1)           # [E, 384]
    msg = sigmoid(z @ Wf + bf) * softplus(z @ Ws + bs)      # [E, 128]
    agg = segment_sum(msg, src, N)                          # [N, 128]
    out = softplus(x + batchnorm(agg))                      # [N, 128]

Strategy — ONE fused SPMD kernel on 8 NeuronCores (the per-dispatch
overhead of this execution stack is ~85 ms, so a single dispatch wins):
  * Host: sort edges by source node, split nodes into 8 contiguous ranges
    with ~equal edge counts.  Each core's range is cut into 128-node
    windows; each window's edges are packed into a FIXED number (TPW) of
    128-edge tiles (padded), so the instruction stream is identical on
    every core (SPMD) while tile contents differ.
  * Everything on device is feature-major ([feat, node/edge]) so the BN
    statistics live in the partition dim as per-partition scalars:
      - per tile: psum[e, 0:256] = z_tile @ [Wf | Ws] via 3 accumulating
        bf16 matmuls; ACT Sigmoid / ACT Softplus; DVE mul -> msg [e, f]
      - scatter: one-hot S[e, n] = (winrel[e] == n); matmul
        psw[f, n] += msg.T @ S accumulates the window's segment-sum in
        PSUM across its TPW tiles (exact fp32 dedup)
      - window epilogue: ACT copy psw -> agg slice with accum_out giving
        the per-feature sum; DVE tensor_tensor_reduce gives sum-of-squares
  * BN stats ([128, 2] per core) are AllReduce'd across the 8 cores
    INSIDE the kernel (gpsimd collective_compute via DRAM bounce), then
    a = gamma*rsqrt(var+eps), b = beta - mean*a as [128,1] columns.
  * Phase 2 (same dispatch): out[f, n] = ACT Softplus(x + ACT(agg*a + b))
    streamed per window; x is preloaded feature-major.
  * Host: de-transpose the 8 output shards into the full [N, 128] output.

The host does only data layout (sort/shard/gather/transpose); all FLOPs,
the segment-sum, the BN reduction and application run on device.
(SWDGE ucode gather/scatter is non-functional under this stack, so edge
gathers are materialized host-side during sharding.)
"""
import sys
sys.path.insert(0, "/opt/trn_rl_repo")

import numpy as np
import ml_dtypes

from concourse import bass, mybir
import concourse.bacc as bacc
import concourse.tile as tile
from concourse.bass_utils import run_bass_kernel_spmd

F32 = mybir.dt.float32
BF16 = mybir.dt.bfloat16

N_CORES = 8
D = 128
BN_EPS = 1e-5

_BUILD_CACHE = {}

# tuning knobs (cache-keyed): phase-1 window group size, pool depths
_WG = 2
_CPOOL_BUFS = 4
_PSE_BUFS = 3
_PSW_BUFS = 2


# --------------------------------------------------------------------------
# host-side packing
# --------------------------------------------------------------------------

def _partition_nodes(src_sorted, n_nodes, n_edges):
    """Split nodes into N_CORES contiguous ranges with ~equal edge counts."""
    deg = np.bincount(src_sorted, minlength=n_nodes)
    cum = np.cumsum(deg)
    bounds = [0]
    for k in range(1, N_CORES):
        bounds.append(int(np.searchsorted(cum, k * n_edges / N_CORES)))
    bounds.append(n_nodes)
    return [int(b) for b in bounds], deg


def _prepare(x, edge_attr, edge_source, edge_target, Wf, bf, Ws, bs, gamma, beta):
    n_nodes, d = x.shape
    n_edges = edge_source.shape[0]
    src = np.asarray(edge_source).astype(np.int64)
    tgt = np.asarray(edge_target).astype(np.int64)

    order = np.argsort(src, kind="stable")
    src_s = src[order]
    bounds, deg = _partition_nodes(src_s, n_nodes, n_edges)
    cum = np.concatenate([[0], np.cumsum(deg)])

    # uniform kernel structure: nw windows x TPW tiles on every core
    nw = max(-(-(bounds[c + 1] - bounds[c]) // 128) for c in range(N_CORES))
    tpw = 1
    for c in range(N_CORES):
        lo, hi = bounds[c], bounds[c + 1]
        for w in range(-(-(hi - lo) // 128)):
            a = lo + w * 128
            b = min(a + 128, hi)
            tpw = max(tpw, -(-int(cum[b] - cum[a]) // 128))
    n_tiles = nw * tpw
    e_pad = n_tiles * 128
    n_pad = nw * 128

    x32 = np.asarray(x, dtype=np.float32)
    ea32 = np.asarray(edge_attr, dtype=np.float32)
    Wf32 = np.asarray(Wf, dtype=np.float32)
    Ws32 = np.asarray(Ws, dtype=np.float32)
    bf32 = np.asarray(bf, dtype=np.float32)
    bs32 = np.asarray(bs, dtype=np.float32)
    with_bias = bool(np.any(bf32) or np.any(bs32))

    # f-block negated so sigmoid(f) = 1/(1 + Exp(-f)) shares the Exp with
    # softplus (ACT has no Sigmoid/Softplus in the Ln/Exp table)
    wsrc = np.concatenate([-Wf32[0:128], Ws32[0:128]], axis=1).astype(ml_dtypes.bfloat16)
    wtgt = np.concatenate([-Wf32[128:256], Ws32[128:256]], axis=1).astype(ml_dtypes.bfloat16)
    wea = np.concatenate([-Wf32[256:384], Ws32[256:384]], axis=1).astype(ml_dtypes.bfloat16)
    bias_row = np.concatenate([-bf32, bs32]).reshape(1, 256).astype(ml_dtypes.bfloat16)

    x16 = x32.astype(ml_dtypes.bfloat16)
    ea16 = ea32.astype(ml_dtypes.bfloat16)
    in_maps = []
    for c in range(N_CORES):
        lo, hi = bounds[c], bounds[c + 1]
        perm = np.full(e_pad, -1, dtype=np.int64)
        winrel = np.full((128, n_tiles), -1.0, dtype=np.float32)
        for w in range(-(-(hi - lo) // 128)):
            a = lo + w * 128
            b = min(a + 128, hi)
            s0, s1 = int(cum[a]), int(cum[b])
            K = s1 - s0
            if K == 0:
                continue
            base = w * tpw * 128
            perm[base:base + K] = order[s0:s1]
            idx = np.arange(K)
            winrel[idx % 128, w * tpw + idx // 128] = (src_s[s0:s1] - a).astype(np.float32)
        mask = perm >= 0
        pm = perm[mask]

        def _featT(rows):
            a_ = np.zeros((e_pad, D), dtype=ml_dtypes.bfloat16)
            a_[mask] = rows
            return np.ascontiguousarray(a_.T)

        srcT = _featT(x16[src[pm]])
        tgtT = _featT(x16[tgt[pm]])
        eaT = _featT(ea16[pm])
        xT = np.zeros((D, n_pad), dtype=np.float32)
        xT[:, 0:hi - lo] = x32[lo:hi].T

        # The per-dispatch runtime cost scales with the NUMBER of IO buffers
        # (~50us per buffer per call on this stack), so pack everything into
        # two mega-tensors. bf16: per-window interleave [ea_w|src_w|tgt_w]
        # (one chunk DMA per window) followed by the weights; f32:
        # [winrel | xT | gamma | beta].
        wb = tpw * 128
        emega = np.empty((D, nw * 3 * wb + 768), dtype=ml_dtypes.bfloat16)
        ev = emega[:, 0:nw * 3 * wb].reshape(D, nw, 3, wb)
        ev[:, :, 0, :] = eaT.reshape(D, nw, wb)
        ev[:, :, 1, :] = srcT.reshape(D, nw, wb)
        ev[:, :, 2, :] = tgtT.reshape(D, nw, wb)
        emega[:, nw * 3 * wb:] = np.concatenate([wsrc, wtgt, wea], axis=1)
        fmega = np.concatenate(
            [winrel, xT,
             np.asarray(gamma, dtype=np.float32).reshape(D, 1),
             np.asarray(beta, dtype=np.float32).reshape(D, 1)], axis=1)
        m = dict(emega=emega, fmega=fmega)
        if with_bias:
            m["bias_row"] = bias_row
        in_maps.append(m)

    meta = dict(nw=nw, tpw=tpw, n_pad=n_pad, bounds=bounds,
                with_bias=with_bias, n_nodes=n_nodes)
    return in_maps, meta


# --------------------------------------------------------------------------
# fused device kernel
# --------------------------------------------------------------------------

def _build_fused(nw, tpw, n_nodes, with_bias, reps=1):
    # reps>1 builds a benchmarking variant that executes the full
    # computation `reps` times back-to-back inside one NEFF (the standard
    # loop-in-kernel pattern, cf. concourse/benchmark/neff_loop.py), so the
    # ~1.5 ms per-dispatch runtime overhead of this stack amortizes away.
    key = ("fused", nw, tpw, n_nodes, with_bias, reps,
           _WG, _CPOOL_BUFS, _PSE_BUFS, _PSW_BUFS)
    if key in _BUILD_CACHE:
        return _BUILD_CACHE[key]
    n_tiles = nw * tpw
    e_pad = n_tiles * 128
    n_pad = nw * 128
    nc = bacc.Bacc(None, debug=False, num_devices=N_CORES)

    wb = tpw * 128
    # two packed inputs: per-dispatch runtime cost scales with IO buffer count
    emega = nc.declare_dram_parameter("emega", [128, nw * 3 * wb + 768], BF16, isOutput=False)
    fmega = nc.declare_dram_parameter("fmega", [128, n_tiles + n_pad + 2], F32, isOutput=False)
    if with_bias:
        bias_row = nc.declare_dram_parameter("bias_row", [1, 256], BF16, isOutput=False)
    outT = nc.declare_dram_parameter("outT", [128, n_pad], F32, isOutput=True)

    AF = mybir.ActivationFunctionType
    with tile.TileContext(nc) as tc:
        with (
            tc.tile_pool(name="res", bufs=1) as res,
            tc.tile_pool(name="chunk", bufs=_CPOOL_BUFS) as cpool,
            tc.tile_pool(name="work", bufs=3) as wpool,
            tc.tile_pool(name="epool", bufs=2 * _WG) as epool,
            tc.tile_pool(name="sppool", bufs=_WG) as sppool,
            tc.tile_pool(name="msgpool", bufs=_WG) as msgpool,
            tc.tile_pool(name="p2pre", bufs=10) as p2pre,
            tc.tile_pool(name="p2ex", bufs=10) as p2ex,
            tc.tile_pool(name="p2ot", bufs=10) as p2ot,
            tc.tile_pool(name="pse", bufs=_PSE_BUFS, space="PSUM") as pse_pool,
            tc.tile_pool(name="psw", bufs=_PSW_BUFS, space="PSUM") as psw_pool,
            tc.tile_pool(name="dram", bufs=2, space="DRAM") as dram,
        ):
            # resident constants / state
            wsb = res.tile([128, 768], BF16, tag="w123")
            nc.sync.dma_start(wsb[:], emega[:, nw * 3 * wb:nw * 3 * wb + 768])
            fsb = res.tile([128, n_tiles + n_pad + 2], F32, tag="fsb")
            nc.sync.dma_start(fsb[:], fmega[:, :])
            X0 = n_tiles           # xT column offset within fsb
            G0 = n_tiles + n_pad   # gamma column, then beta column
            iota_i = res.tile([128, 128], mybir.dt.int32, tag="ii")
            nc.gpsimd.iota(iota_i[:], pattern=[[1, 128]], base=0, channel_multiplier=0)
            iota_f = res.tile([128, 128], F32, tag="if")
            nc.vector.tensor_copy(iota_f[:], iota_i[:])
            if with_bias:
                ones_bf = res.tile([1, 128], BF16, tag="ob")
                nc.vector.memset(ones_bf[:], 1.0)
                brow = res.tile([1, 256], BF16, tag="br")
                nc.sync.dma_start(brow[:], bias_row[:, :])

            agg_sb = res.tile([128, n_pad], F32, tag="agg")
            stats_sum = res.tile([128, nw], F32, tag="ssum")
            stats_sq = res.tile([128, nw], F32, tag="ssq")

            for rep in range(reps):
                # ---------------- phase 1: edge messages + segment sum ----------
                # staged in groups of WG windows: all matmul+Exp first, then all
                # Lns, then the DVE/scatter chain. Keeps the ACT queue's Exp/Ln
                # runs contiguous: Exp and Ln live in different ACT function
                # tables, and interleaving them costs a ~1.3us table reload per
                # switch (the profile showed 664 reloads = 62% of kernel time
                # when naively interleaved).
                WG = _WG
                for wg0 in range(0, nw, WG):
                    wgs = range(wg0, min(wg0 + WG, nw))
                    chunks = {}
                    for w in wgs:
                        # one DMA per window: [ea_w | src_w | tgt_w] interleaved
                        ch = cpool.tile([128, 3 * wb], BF16, tag="chunk")
                        nc.sync.dma_start(ch[:], emega[:, w * 3 * wb:(w + 1) * 3 * wb])
                        chunks[w] = ch
                    es = {}
                    for w in wgs:
                        ch = chunks[w]
                        # split Exp into f/s halves written window-contiguous so
                        # the softplus Ln and the sigmoid DVE chain each run as
                        # ONE wide instruction per window (instruction fixed
                        # overhead dominates 128-wide DVE/ACT ops)
                        ef_w = epool.tile([128, wb], F32, tag="ef")
                        es_w = epool.tile([128, wb], F32, tag="es")
                        for j in range(tpw):
                            j0 = j * 128
                            ps_e = pse_pool.tile([128, 256], F32, tag="pse")
                            nc.tensor.matmul(ps_e[:], lhsT=ch[:, wb + j0:wb + j0 + 128],
                                             rhs=wsb[:, 0:256], start=True, stop=False)
                            nc.tensor.matmul(ps_e[:], lhsT=ch[:, 2 * wb + j0:2 * wb + j0 + 128],
                                             rhs=wsb[:, 256:512], start=False, stop=False)
                            nc.tensor.matmul(ps_e[:], lhsT=ch[:, j0:j0 + 128],
                                             rhs=wsb[:, 512:768], start=False,
                                             stop=not with_bias)
                            if with_bias:
                                nc.tensor.matmul(ps_e[:], lhsT=ones_bf[:], rhs=brow[:],
                                                 start=False, stop=True)
                            nc.scalar.activation(ef_w[:, j0:j0 + 128], ps_e[:, 0:128], AF.Exp)
                            nc.scalar.activation(es_w[:, j0:j0 + 128], ps_e[:, 128:256], AF.Exp)
                        es[w] = (ef_w, es_w)
                    sps = {}
                    for w in wgs:
                        # sp = Ln(e_s + 1)   (softplus), whole window at once
                        sp_w = sppool.tile([128, wb], F32, tag="sp")
                        nc.scalar.activation(sp_w[:], es[w][1][:], AF.Ln, bias=1.0)
                        sps[w] = sp_w
                    for w in wgs:
                        ef_w, _ = es[w]
                        # sig = 1/(1+e_f);  msg = sig * sp  — one wide op each
                        nc.vector.tensor_scalar_add(ef_w[:], ef_w[:], 1.0)
                        nc.vector.reciprocal(ef_w[:], ef_w[:])
                        msg_w = msgpool.tile([128, wb], BF16, tag="msg")
                        nc.vector.tensor_mul(msg_w[:], ef_w[:], sps[w][:])
                        psw = psw_pool.tile([128, 128], F32, tag="psw")
                        for j in range(tpw):
                            t = w * tpw + j
                            j0 = j * 128
                            S = wpool.tile([128, 128], BF16, tag="S")
                            nc.vector.tensor_tensor(
                                out=S[:],
                                in0=fsb[:, t:t + 1].to_broadcast([128, 128]),
                                in1=iota_f[:],
                                op=mybir.AluOpType.is_equal,
                            )
                            # psw[f, n] += msg.T @ S  (segment-sum of the window)
                            nc.tensor.matmul(psw[:], lhsT=msg_w[:, j0:j0 + 128], rhs=S[:],
                                             start=(j == 0), stop=(j == tpw - 1),
                                             skip_group_check=True)
                        wsl = slice(w * 128, (w + 1) * 128)
                        # copy psum -> agg slice; accum_out = per-feature sum
                        nc.scalar.activation(agg_sb[:, wsl], psw[:], AF.Identity,
                                             accum_out=stats_sum[:, w:w + 1])
                        # sum of squares via ACT Square + accum_out (the DVE
                        # tensor_tensor_reduce path crashes the device here)
                        sq = wpool.tile([128, 128], F32, tag="sq")
                        nc.scalar.activation(sq[:], psw[:], AF.Square,
                                             accum_out=stats_sq[:, w:w + 1])

                # ---------------- BN statistics + cross-core AllReduce ----------
                st2 = res.tile([128, 2], F32, tag="st2")
                nc.vector.tensor_reduce(st2[:, 0:1], stats_sum[:],
                                        mybir.AxisListType.X, mybir.AluOpType.add)
                nc.vector.tensor_reduce(st2[:, 1:2], stats_sq[:],
                                        mybir.AxisListType.X, mybir.AluOpType.add)
                cin = dram.tile([128, 2], F32)
                cout = dram.tile([128, 2], F32)
                nc.gpsimd.dma_start(cin[:], st2[:])
                nc.gpsimd.collective_compute(
                    "AllReduce", mybir.AluOpType.add,
                    replica_groups=[list(range(N_CORES))],
                    ins=[cin.opt()], outs=[cout.opt()],
                )
                stg = res.tile([128, 2], F32, tag="stg")
                nc.gpsimd.dma_start(stg[:], cout[:])

                mean = res.tile([128, 1], F32, tag="mean")
                nc.vector.tensor_scalar_mul(mean[:], stg[:, 0:1], 1.0 / n_nodes)
                ex2 = res.tile([128, 1], F32, tag="ex2")
                nc.vector.tensor_scalar_mul(ex2[:], stg[:, 1:2], 1.0 / n_nodes)
                m2 = res.tile([128, 1], F32, tag="m2")
                nc.vector.tensor_mul(m2[:], mean[:], mean[:])
                var = res.tile([128, 1], F32, tag="var")
                nc.vector.tensor_sub(var[:], ex2[:], m2[:])
                nc.vector.tensor_scalar_add(var[:], var[:], BN_EPS)
                # rsqrt(v) = Exp(-0.5 * Ln(v)) — keeps everything in one ACT table
                lnv = res.tile([128, 1], F32, tag="lnv")
                nc.scalar.activation(lnv[:], var[:], AF.Ln)
                inv = res.tile([128, 1], F32, tag="inv")
                nc.scalar.activation(inv[:], lnv[:], AF.Exp, scale=-0.5)
                a_col = res.tile([128, 1], F32, tag="acol")
                nc.vector.tensor_mul(a_col[:], inv[:], fsb[:, G0:G0 + 1])
                ma = res.tile([128, 1], F32, tag="ma")
                nc.vector.tensor_mul(ma[:], mean[:], a_col[:])
                b_col = res.tile([128, 1], F32, tag="bcol")
                nc.vector.tensor_sub(b_col[:], fsb[:, G0 + 1:G0 + 2], ma[:])

                # ---------------- phase 2: BN apply + softplus -----------------
                # batched in groups so the ACT queue sees runs of the same
                # function (Identity*G, Exp*G, Ln*G) -> 2 table loads per group
                G = 10
                for w0 in range(0, nw, G):
                    ws = range(w0, min(w0 + G, nw))
                    pres, exs = [], []
                    for w in ws:
                        wsl = slice(w * 128, (w + 1) * 128)
                        pre = p2pre.tile([128, 128], F32, tag="pre")
                        nc.scalar.activation(pre[:], agg_sb[:, wsl], AF.Identity,
                                             bias=b_col[:], scale=a_col[:])
                        nc.vector.tensor_add(pre[:], pre[:],
                                             fsb[:, X0 + w * 128:X0 + (w + 1) * 128])
                        pres.append(pre)
                    for i, w in enumerate(ws):
                        ex = p2ex.tile([128, 128], F32, tag="ex")
                        nc.scalar.activation(ex[:], pres[i][:], AF.Exp)
                        exs.append(ex)
                    for i, w in enumerate(ws):
                        wsl = slice(w * 128, (w + 1) * 128)
                        ot = p2ot.tile([128, 128], F32, tag="ot")
                        nc.scalar.activation(ot[:], exs[i][:], AF.Ln, bias=1.0)
                        nc.sync.dma_start(outT[:, wsl], ot[:])

    nc.compile()
    _BUILD_CACHE[key] = nc
    return nc


# --------------------------------------------------------------------------
# entry point
# --------------------------------------------------------------------------

def kernel(x, edge_attr, edge_source, edge_target, Wf, bf, Ws, bs, gamma, beta):
    x = np.asarray(x)
    n_nodes = x.shape[0]
    in_maps, meta = _prepare(x, edge_attr, edge_source, edge_target,
                             Wf, bf, Ws, bs, gamma, beta)
    nc = _build_fused(meta["nw"], meta["tpw"], meta["n_nodes"], meta["with_bias"])
    res = run_bass_kernel_spmd(nc, in_maps, core_ids=list(range(N_CORES)))

    bounds = meta["bounds"]
    out = np.empty((n_nodes, 128), dtype=np.float32)
    for c in range(N_CORES):
        lo, hi = bounds[c], bounds[c + 1]
        out[lo:hi] = np.asarray(res.results[c]["outT"])[:, 0:hi - lo].T
    return out



# revision 61
# speedup vs baseline: 2.1699x; 2.1699x over previous
"""Trainium2 Bass kernel for nn_Node_Convolution (GNN message passing).

Reference computation:
    z   = concat([x[src], x[tgt], edge_attr], -1)           # [E, 384]
    msg = sigmoid(z @ Wf + bf) * softplus(z @ Ws + bs)      # [E, 128]
    agg = segment_sum(msg, src, N)                          # [N, 128]
    out = softplus(x + batchnorm(agg))                      # [N, 128]

Strategy — ONE fused SPMD kernel on 8 NeuronCores (the per-dispatch
overhead of this execution stack is ~85 ms, so a single dispatch wins):
  * Host: sort edges by source node, split nodes into 8 contiguous ranges
    with ~equal edge counts.  Each core's range is cut into 128-node
    windows; each window's edges are packed into a FIXED number (TPW) of
    128-edge tiles (padded), so the instruction stream is identical on
    every core (SPMD) while tile contents differ.
  * Everything on device is feature-major ([feat, node/edge]) so the BN
    statistics live in the partition dim as per-partition scalars.
  * Per window: 3 accumulating bf16 matmuls per 128-edge tile into PSUM
    (4 tiles packed per 2-bank psum group), ONE wide ACT Exp per psum
    group, ONE wide ACT Ln (softplus), a short wide bf16 DVE chain
    (+1 / reciprocal / multiply) for the sigmoid, a single wide Pool
    (gpsimd) is_equal building ALL the window's one-hot scatter matrices,
    then per-tile scatter matmuls accumulating the window's segment-sum
    in PSUM (exact fp32).  The f-block of the fused weight is negated so
    sigmoid(f) = 1/(1 + Exp(-f)) shares the Exp pass with softplus: every
    activation stays in the single natural_log_exp_and_others ACT table
    (the insertion pass is steered to it; see _build_fused), so there are
    no per-window table reloads.
  * BN stats ([128, 2] per core: sum, sum-of-squares over the node dim)
    are two wide DVE ops over the fp32 agg, AllReduce'd across the 8
    cores INSIDE the kernel (gpsimd collective_compute via DRAM bounce),
    then a = gamma*rsqrt(var+eps), b = beta - mean*a as [128,1] columns.
  * Phase 2 (same dispatch): out = Ln(1 + Exp(agg*a + (x + b))) computed
    as 2 full-width DVE ops + 2 full-width ACT ops; bf16 output.
  * Host: de-transpose the 8 output shards into the full [N, 128] output.

The host does only data layout (sort/shard/gather/transpose); all FLOPs,
the segment-sum, the BN reduction and application run on device.
(SWDGE ucode gather/scatter is non-functional under this stack, so edge
gathers are materialized host-side during sharding.)
"""
import sys
sys.path.insert(0, "/opt/trn_rl_repo")

import numpy as np
import ml_dtypes

from concourse import bass, mybir
import concourse.bacc as bacc
import concourse.tile as tile
from concourse.bass_utils import run_bass_kernel_spmd
from concourse.hw_specs import get_activation_tables
import bass_rust as _bass_rust

F32 = mybir.dt.float32
BF16 = mybir.dt.bfloat16

N_CORES = 8
D = 128
BN_EPS = 1e-5

_BUILD_CACHE = {}

# tuning knobs (cache-keyed): psum group width (edge tiles per ACT Exp),
# pool depths
_PSG = 4
_CPOOL_BUFS = 3
_PSE_BUFS = 2
_RS = 4     # windows with rep-resident one-hot S tiles
_KB = 1     # windows using the ACT-side sigmoid recipe (DVE<->ACT balance)
_DEFER = 3  # windows of scatter deferral (hides the ACT+DVE chain latency)


# --------------------------------------------------------------------------
# host-side packing
# --------------------------------------------------------------------------

def _partition_nodes(src_sorted, n_nodes, n_edges):
    """Split nodes into N_CORES contiguous ranges with ~equal edge counts."""
    deg = np.bincount(src_sorted, minlength=n_nodes)
    cum = np.cumsum(deg)
    bounds = [0]
    for k in range(1, N_CORES):
        bounds.append(int(np.searchsorted(cum, k * n_edges / N_CORES)))
    bounds.append(n_nodes)
    return [int(b) for b in bounds], deg


def _pack_windows(degs, nw):
    """Assign local nodes to nw windows (<=128 nodes each), balancing edge
    counts (greedy LPT).  Returns (assign, widx): window and in-window index
    per local node."""
    import heapq
    n = len(degs)
    assign = np.empty(n, dtype=np.int64)
    widx = np.empty(n, dtype=np.int64)
    heap = [(0, 0, w) for w in range(nw)]
    heapq.heapify(heap)
    for ln in np.argsort(-degs, kind="stable"):
        while True:
            e, cnt, w = heapq.heappop(heap)
            if cnt < 128:
                break
            # window full: drop it from the heap for good
        assign[ln] = w
        widx[ln] = cnt
        heapq.heappush(heap, (e + int(degs[ln]), cnt + 1, w))
    return assign, widx


def _prepare(x, edge_attr, edge_source, edge_target, Wf, bf, Ws, bs, gamma, beta):
    n_nodes, d = x.shape
    n_edges = edge_source.shape[0]
    src = np.asarray(edge_source).astype(np.int64)
    tgt = np.asarray(edge_target).astype(np.int64)

    order = np.argsort(src, kind="stable")
    src_s = src[order]
    bounds, deg = _partition_nodes(src_s, n_nodes, n_edges)
    cum = np.concatenate([[0], np.cumsum(deg)])

    # uniform kernel structure: nw windows x TPW tiles on every core.
    # Nodes are BIN-PACKED into windows (balanced edge counts) so tpw =
    # ceil(max window edge count / 128) stays near the global average
    # instead of the worst contiguous 128-node run.
    nw = max(-(-(bounds[c + 1] - bounds[c]) // 128) for c in range(N_CORES))
    packs = []
    tpw = 1
    for c in range(N_CORES):
        lo, hi = bounds[c], bounds[c + 1]
        assign, widx = _pack_windows(deg[lo:hi], nw)
        wedges = np.bincount(assign, weights=deg[lo:hi], minlength=nw).astype(np.int64)
        tpw = max(tpw, -(-int(wedges.max()) // 128))
        packs.append((assign, widx))
    n_tiles = nw * tpw
    e_pad = n_tiles * 128
    n_pad = nw * 128

    x32 = np.asarray(x, dtype=np.float32)
    ea32 = np.asarray(edge_attr, dtype=np.float32)
    Wf32 = np.asarray(Wf, dtype=np.float32)
    Ws32 = np.asarray(Ws, dtype=np.float32)
    bf32 = np.asarray(bf, dtype=np.float32)
    bs32 = np.asarray(bs, dtype=np.float32)
    with_bias = bool(np.any(bf32) or np.any(bs32))

    # f-block negated so sigmoid(f) = 1/(1 + Exp(-f)) shares the Exp with
    # softplus (keeps every ACT call in the natural_log_exp_and_others table)
    wsrc = np.concatenate([-Wf32[0:128], Ws32[0:128]], axis=1).astype(ml_dtypes.bfloat16)
    wtgt = np.concatenate([-Wf32[128:256], Ws32[128:256]], axis=1).astype(ml_dtypes.bfloat16)
    wea = np.concatenate([-Wf32[256:384], Ws32[256:384]], axis=1).astype(ml_dtypes.bfloat16)
    bias_row = np.concatenate([-bf32, bs32]).reshape(1, 256).astype(ml_dtypes.bfloat16)

    x16 = x32.astype(ml_dtypes.bfloat16)
    ea16 = ea32.astype(ml_dtypes.bfloat16)
    in_maps = []
    node_of_col = []
    for c in range(N_CORES):
        lo, hi = bounds[c], bounds[c + 1]
        assign, widx = packs[c]
        # per-window edge lists: concat of member nodes' (sorted) edge spans
        perm = np.full(e_pad, -1, dtype=np.int64)
        winrel = np.full((128, n_tiles), -1.0, dtype=np.float32)
        noc = np.full(n_pad, -1, dtype=np.int64)
        for w in range(nw):
            members = np.nonzero(assign == w)[0]
            members = members[np.argsort(widx[members])]
            noc[w * 128 + widx[members]] = lo + members
            if len(members) == 0:
                continue
            # edge indices (into the src-sorted order) and their winrel
            spans = [order[cum[lo + ln]:cum[lo + ln + 1]] for ln in members]
            reps_ = [cum[lo + ln + 1] - cum[lo + ln] for ln in members]
            ew = np.concatenate(spans) if spans else np.empty(0, np.int64)
            K = len(ew)
            if K == 0:
                continue
            wr = np.repeat(widx[members], reps_).astype(np.float32)
            base = w * tpw * 128
            perm[base:base + K] = ew
            idx = np.arange(K)
            winrel[idx % 128, w * tpw + idx // 128] = wr
        mask = perm >= 0
        pm = perm[mask]

        def _featT(rows):
            a_ = np.zeros((e_pad, D), dtype=ml_dtypes.bfloat16)
            a_[mask] = rows
            return np.ascontiguousarray(a_.T)

        srcT = _featT(x16[src[pm]])
        tgtT = _featT(x16[tgt[pm]])
        eaT = _featT(ea16[pm])
        xT = np.zeros((D, n_pad), dtype=ml_dtypes.bfloat16)
        valid = noc >= 0
        xT[:, valid] = x16[noc[valid]].T
        node_of_col.append(noc)

        # The per-dispatch runtime cost scales with the NUMBER of IO buffers
        # (~50us per buffer per call on this stack), so pack everything into
        # two mega-tensors. bf16: per-window interleave [ea_w|src_w|tgt_w]
        # (one chunk DMA per window), then the weights, xT, winrel; f32:
        # [gamma | beta].
        wb = tpw * 128
        ecols = nw * 3 * wb
        emega = np.empty((D, ecols + 768 + n_pad + n_tiles), dtype=ml_dtypes.bfloat16)
        ev = emega[:, 0:ecols].reshape(D, nw, 3, wb)
        ev[:, :, 0, :] = eaT.reshape(D, nw, wb)
        ev[:, :, 1, :] = srcT.reshape(D, nw, wb)
        ev[:, :, 2, :] = tgtT.reshape(D, nw, wb)
        emega[:, ecols:ecols + 768] = np.concatenate([wsrc, wtgt, wea], axis=1)
        emega[:, ecols + 768:ecols + 768 + n_pad] = xT
        emega[:, ecols + 768 + n_pad:] = winrel.astype(ml_dtypes.bfloat16)
        fmega = np.stack([np.asarray(gamma, dtype=np.float32),
                          np.asarray(beta, dtype=np.float32)], axis=1)
        m = dict(emega=emega, fmega=fmega)
        if with_bias:
            m["bias_row"] = bias_row
        in_maps.append(m)

    meta = dict(nw=nw, tpw=tpw, n_pad=n_pad, bounds=bounds,
                with_bias=with_bias, n_nodes=n_nodes,
                node_of_col=node_of_col)
    return in_maps, meta


# --------------------------------------------------------------------------
# fused device kernel
# --------------------------------------------------------------------------

def _steer_act_tables(nc):
    """Steer the act-table insertion pass to natural_log_exp_and_others.

    The stock pass picks, per activation function, the first table that
    contains it (exp -> exp_and_others, ln -> natural_log), which makes an
    Exp/Ln-alternating kernel reload tables ~60x (~2.7us each).  Reordering
    the table list so the combined exp+ln table comes first makes the
    fixpoint settle on ONE load; the emitted ids (indexes into the passed
    list) are then remapped to the real act_info.json order.
    """
    orig = nc.insert_act_table_loads

    def patched():
        tabs = get_activation_tables(nc.m.arch)
        names = list(tabs)
        pref = "natural_log_exp_and_others"
        ordered = [(pref, tabs[pref])] + [
            (n, s) for n, s in tabs.items() if n != pref
        ]
        has_activation = any(
            isinstance(i, mybir.InstActivation)
            for b in nc.main_func.blocks
            for i in b.instructions
        )
        if not has_activation:
            return
        _bass_rust.insert_act_table_loads(nc, list(ordered))
        remap = {i: names.index(n) for i, (n, _) in enumerate(ordered)}
        for b in nc.main_func.blocks:
            for inst in b.instructions:
                if isinstance(inst, mybir.InstLoadActFuncSet):
                    inst.act_func_set_id = remap[inst.act_func_set_id]

    nc.insert_act_table_loads = patched
    return orig


def _build_fused(nw, tpw, n_nodes, with_bias, reps=1):
    # reps>1 builds a benchmarking variant that executes the full
    # computation `reps` times back-to-back inside one NEFF (the standard
    # loop-in-kernel pattern, cf. concourse/benchmark/neff_loop.py), so the
    # ~1.5 ms per-dispatch runtime overhead of this stack amortizes away.
    key = ("fused2", nw, tpw, n_nodes, with_bias, reps,
           _PSG, _CPOOL_BUFS, _PSE_BUFS, _RS, _KB, _DEFER)
    if key in _BUILD_CACHE:
        return _BUILD_CACHE[key]
    n_tiles = nw * tpw
    n_pad = nw * 128
    wb = tpw * 128
    ecols = nw * 3 * wb
    nc = bacc.Bacc(None, debug=False, num_devices=N_CORES)

    emega = nc.declare_dram_parameter(
        "emega", [128, ecols + 768 + n_pad + n_tiles], BF16, isOutput=False)
    fmega = nc.declare_dram_parameter("fmega", [128, 2], F32, isOutput=False)
    if with_bias:
        bias_row = nc.declare_dram_parameter("bias_row", [1, 256], BF16, isOutput=False)
    outT = nc.declare_dram_parameter("outT", [128, n_pad], BF16, isOutput=True)

    AF = mybir.ActivationFunctionType
    ALU = mybir.AluOpType
    # psum edge-tile groups per window: _PSG tiles (2 banks) per group
    groups = [(g0, min(g0 + _PSG, tpw)) for g0 in range(0, tpw, _PSG)]

    with tile.TileContext(nc) as tc:
        with (
            tc.tile_pool(name="res", bufs=1) as res,
            tc.tile_pool(name="chunk", bufs=_CPOOL_BUFS) as cpool,
            tc.tile_pool(name="eu", bufs=3) as eupool,
            tc.tile_pool(name="sp", bufs=3) as sppool,
            tc.tile_pool(name="ad", bufs=2) as adpool,
            tc.tile_pool(name="msg", bufs=_DEFER + 1) as msgpool,
            tc.tile_pool(name="sall", bufs=_DEFER + 1) as spool,
            tc.tile_pool(name="pse", bufs=_PSE_BUFS, space="PSUM") as pse_pool,
            tc.tile_pool(name="psw", bufs=_DEFER, space="PSUM") as psw_pool,
            tc.tile_pool(name="dram", bufs=2, space="DRAM") as dram,
        ):
            # resident constants / state
            wsb = res.tile([128, 768], BF16, tag="w123")
            nc.sync.dma_start(wsb[:], emega[:, ecols:ecols + 768])
            xsb = res.tile([128, n_pad], BF16, tag="xsb")
            nc.sync.dma_start(xsb[:], emega[:, ecols + 768:ecols + 768 + n_pad])
            wr_sb = res.tile([128, n_tiles], BF16, tag="wr")
            nc.sync.dma_start(wr_sb[:], emega[:, ecols + 768 + n_pad:])
            gb = res.tile([128, 2], F32, tag="gb")
            nc.sync.dma_start(gb[:], fmega[:, :])
            # iota repeated per tile: [128, tpw, 128] with value = inner index
            iota_i = res.tile([128, tpw, 128], mybir.dt.int32, tag="ii")
            nc.gpsimd.iota(iota_i[:], pattern=[[0, tpw], [1, 128]], base=0,
                           channel_multiplier=0)
            iota_w = res.tile([128, tpw, 128], BF16, tag="iw")
            nc.vector.tensor_copy(iota_w[:], iota_i[:])
            if with_bias:
                ones_bf = res.tile([1, 128], BF16, tag="ob")
                nc.vector.memset(ones_bf[:], 1.0)
                brow = res.tile([1, 256], BF16, tag="br")
                nc.sync.dma_start(brow[:], bias_row[:, :])

            # agg ping-pongs between reps so the deferred phase 2 of rep r
            # (emitted inside rep r+1's phase 1) never WARs with rep r+1's
            # evictions
            agg0 = res.tile([128, n_pad], F32, tag="agg0")
            agg1 = res.tile([128, n_pad], F32, tag="agg1")
            aggs = [agg0, agg1]
            eo = res.tile([128, n_pad], BF16, tag="eo")
            st2 = res.tile([128, 2], F32, tag="st2")
            stg = res.tile([128, 2], F32, tag="stg")

            def scatter_window(w, msg, s_all, agg):
                # scatter: psw[f, n] += msg_j.T @ S_j across the window, then
                # evict psum -> agg slice
                psw = psw_pool.tile([128, 128], F32, tag="psw")
                for j in range(tpw):
                    nc.tensor.matmul(psw[:], lhsT=msg[:, j, :], rhs=s_all[:, j, :],
                                     start=(j == 0), stop=(j == tpw - 1),
                                     skip_group_check=True)
                nc.vector.tensor_copy(agg[:, w * 128:(w + 1) * 128], psw[:])

            def build_s(w, pool=None, tag="sall"):
                # one wide one-hot build for the whole window (DVE; walrus
                # rejects comparison ALU ops on Pool)
                s_all = (pool or spool).tile([128, tpw, 128], BF16, tag=tag)
                nc.vector.tensor_tensor(
                    out=s_all[:],
                    in0=wr_sb[:, w * tpw:(w + 1) * tpw].to_broadcast(
                        [128, tpw, 128]),
                    in1=iota_w[:],
                    op=ALU.is_equal,
                )
                return s_all

            def emit_bn_consts(rep):
                # BN constants from the allreduced stats of `rep`
                mean = res.tile([128, 1], F32, tag="mean")
                nc.vector.tensor_scalar_mul(mean[:], stg[:, 0:1], 1.0 / n_nodes)
                ex2 = res.tile([128, 1], F32, tag="ex2")
                nc.vector.tensor_scalar_mul(ex2[:], stg[:, 1:2], 1.0 / n_nodes)
                m2 = res.tile([128, 1], F32, tag="m2")
                nc.vector.tensor_mul(m2[:], mean[:], mean[:])
                var = res.tile([128, 1], F32, tag="var")
                nc.vector.tensor_sub(var[:], ex2[:], m2[:])
                nc.vector.tensor_scalar_add(var[:], var[:], BN_EPS)
                # rsqrt(v) = Exp(-0.5 * Ln(v)) — stays in the one ACT table
                lnv = res.tile([128, 1], F32, tag="lnv")
                nc.scalar.activation(lnv[:], var[:], AF.Ln)
                inv = res.tile([128, 1], F32, tag="inv")
                nc.scalar.activation(inv[:], lnv[:], AF.Exp, scale=-0.5)
                a_col = res.tile([128, 1], F32, tag="acol")
                nc.vector.tensor_mul(a_col[:], inv[:], gb[:, 0:1])
                ma = res.tile([128, 1], F32, tag="ma")
                nc.vector.tensor_mul(ma[:], mean[:], a_col[:])
                b_col = res.tile([128, 1], F32, tag="bcol")
                nc.vector.tensor_sub(b_col[:], gb[:, 1:2], ma[:])
                return a_col, b_col

            _NSLAB = 4
            _SLABW = n_pad // _NSLAB

            def emit_phase2_slab(rep, cols, s):
                # phase 2 for `rep`: agg = agg*a + x (fused STT), then
                # out = Ln(1 + Exp(agg + b)) with b folded into the ACT Exp
                # bias.  Output DMA on the ACT queue, not sync (sync streams
                # the next rep's chunks).
                a_col, b_col = cols
                agg = aggs[rep % 2]
                sl = slice(s * _SLABW, (s + 1) * _SLABW if s < _NSLAB - 1 else n_pad)
                nc.vector.scalar_tensor_tensor(
                    out=agg[:, sl], in0=agg[:, sl], scalar=a_col[:],
                    in1=xsb[:, sl], op0=ALU.mult, op1=ALU.add)
                nc.scalar.activation(eo[:, sl], agg[:, sl], AF.Exp, bias=b_col[:])
                nc.scalar.activation(eo[:, sl], eo[:, sl], AF.Ln, bias=1.0)
                nc.scalar.dma_start(outT[:, sl], eo[:, sl])

            # S tiles are rep-invariant; the first _RS windows' S tiles stay
            # RESIDENT across reps (like the weights), so the next rep's
            # scatters never wait on S rebuilt behind the stats/collective.
            s_res = {w: build_s(w, pool=res, tag=f"sres{w}")
                     for w in range(min(_RS, nw))}

            # Rep r's BN consts + phase 2 are deferred INTO rep r+1's window
            # loop (by then the AllReduce has completed), so no engine queue
            # ever head-of-line blocks on the collective.
            pend_bn = None     # rep whose BN/phase2 is still to emit
            bn_cols = None

            for rep in range(reps):
                # ---------------- phase 1: edge messages + segment sum ------
                # Window w's scatter matmuls depend on its full ACT+DVE chain;
                # issuing them AFTER window w+1's z-matmuls keeps the (FIFO)
                # PE queue from stalling on the chain (software pipelining).
                agg = aggs[rep % 2]
                pend = []
                for w in range(nw):
                    # one DMA per window: [ea_w | src_w | tgt_w] interleaved
                    ch = cpool.tile([128, 3 * wb], BF16, tag="chunk")
                    nc.sync.dma_start(ch[:], emega[:, w * 3 * wb:(w + 1) * 3 * wb])

                    s_all = s_res.get(w)
                    if s_all is None:
                        s_all = build_s(w)

                    # matmuls: _PSG edge tiles per 2/3-bank psum group, then
                    # one wide Exp per group into the window's EU buffer
                    eu = eupool.tile([128, tpw, 256], BF16, tag="eu")
                    for g0, g1 in groups:
                        ps = pse_pool.tile([128, _PSG, 256], F32, tag="pse")
                        for j in range(g0, g1):
                            jj = j - g0
                            # bank-wide accumulation groups: 2 tiles per bank
                            first = jj % 2 == 0
                            last_in_bank = (jj % 2 == 1) or (j == g1 - 1)
                            nc.tensor.matmul(ps[:, jj, :],
                                             lhsT=ch[:, wb + j * 128:wb + j * 128 + 128],
                                             rhs=wsb[:, 0:256],
                                             start=first, stop=False)
                            nc.tensor.matmul(ps[:, jj, :],
                                             lhsT=ch[:, 2 * wb + j * 128:2 * wb + j * 128 + 128],
                                             rhs=wsb[:, 256:512],
                                             start=False, stop=False)
                            nc.tensor.matmul(ps[:, jj, :],
                                             lhsT=ch[:, j * 128:j * 128 + 128],
                                             rhs=wsb[:, 512:768],
                                             start=False,
                                             stop=last_in_bank and not with_bias)
                            if with_bias:
                                nc.tensor.matmul(ps[:, jj, :], lhsT=ones_bf[:],
                                                 rhs=brow[:],
                                                 start=False, stop=last_in_bank)
                        nc.scalar.activation(eu[:, g0:g1, :], ps[:, 0:g1 - g0, :],
                                             AF.Exp)

                    msg = msgpool.tile([128, tpw, 128], BF16, tag="msg")
                    if w < _KB:
                        # ACT-side sigmoid (balances DVE->ACT): one Ln over
                        # BOTH halves gives [L | softplus] with
                        # L = Ln(1+e^-f) = -Ln(sigmoid); sigmoid = Exp(-L)
                        lu = eupool.tile([128, tpw, 256], BF16, tag="lu")
                        nc.scalar.activation(lu[:], eu[:], AF.Ln, bias=1.0)
                        sg = adpool.tile([128, tpw, 128], BF16, tag="sg")
                        nc.scalar.activation(sg[:], lu[:, :, 0:128], AF.Exp,
                                             scale=-1.0)
                        nc.vector.tensor_tensor(out=msg[:],
                                                in0=lu[:, :, 128:256],
                                                in1=sg[:], op=ALU.mult)
                    else:
                        # DVE-side sigmoid: softplus = Ln(1 + e_s) on ACT;
                        # sigmoid = 1/(1 + e_f) on DVE via the fast-approx
                        # reciprocal (fp32 bit trick + 2 Newton steps, ~18
                        # correct bits, ~7x faster than the iterative
                        # nc.vector.reciprocal on HW).  Input is in (1, ~500]
                        # so the approx's 0/denorm/inf edge cases can't occur.
                        sp = sppool.tile([128, tpw, 128], BF16, tag="sp")
                        nc.scalar.activation(sp[:], eu[:, :, 128:256], AF.Ln,
                                             bias=1.0)
                        ad = adpool.tile([128, tpw, 128], F32, tag="ad")
                        nc.vector.tensor_scalar_add(ad[:], eu[:, :, 0:128], 1.0)
                        rc = adpool.tile([128, tpw, 128], F32, tag="rc")
                        nc.vector.reciprocal_approx_fast(rc[:], ad[:])
                        sg = adpool.tile([128, tpw, 128], BF16, tag="sgc")
                        nc.vector.tensor_copy(sg[:], rc[:])
                        nc.vector.tensor_tensor(out=msg[:], in0=sp[:], in1=sg[:],
                                                op=ALU.mult)

                    pend.append((w, msg, s_all, agg))
                    if len(pend) > _DEFER:
                        scatter_window(*pend.pop(0))

                    # deferred BN consts + phase 2 of the PREVIOUS rep,
                    # interleaved into this rep's phase 1
                    if pend_bn is not None:
                        if w == 8:
                            bn_cols = emit_bn_consts(pend_bn)
                        elif w in (10, 12, 14, 16):
                            emit_phase2_slab(pend_bn, bn_cols, (w - 10) // 2)
                            if w == 16:
                                pend_bn = None
                for p in pend:
                    scatter_window(*p)

                # ---------------- BN statistics + cross-core AllReduce ------
                nc.vector.tensor_reduce(st2[:, 0:1], agg[:],
                                        mybir.AxisListType.X, ALU.add)
                # sum of squares on ACT (Square + accum_out); the bf16 eo
                # scratch is write-only
                nc.scalar.activation(eo[:], agg[:], AF.Square,
                                     accum_out=st2[:, 1:2])
                # bounce DMAs stay OFF the sync queue (it streams the next
                # rep's chunks)
                cin = dram.tile([128, 2], F32)
                cout = dram.tile([128, 2], F32)
                nc.scalar.dma_start(cin[:], st2[:])
                nc.gpsimd.collective_compute(
                    "AllReduce", ALU.add,
                    replica_groups=[list(range(N_CORES))],
                    ins=[cin.opt()], outs=[cout.opt()],
                )
                nc.gpsimd.dma_start(stg[:], cout[:])
                pend_bn = rep

            # tail: BN + phase 2 of the final rep
            bn_cols = emit_bn_consts(pend_bn)
            for s in range(_NSLAB):
                emit_phase2_slab(pend_bn, bn_cols, s)

    _steer_act_tables(nc)
    nc.compile()
    _BUILD_CACHE[key] = nc
    return nc


# --------------------------------------------------------------------------
# entry point
# --------------------------------------------------------------------------

def kernel(x, edge_attr, edge_source, edge_target, Wf, bf, Ws, bs, gamma, beta):
    x = np.asarray(x)
    n_nodes = x.shape[0]
    in_maps, meta = _prepare(x, edge_attr, edge_source, edge_target,
                             Wf, bf, Ws, bs, gamma, beta)
    nc = _build_fused(meta["nw"], meta["tpw"], meta["n_nodes"], meta["with_bias"])
    res = run_bass_kernel_spmd(nc, in_maps, core_ids=list(range(N_CORES)))

    out = np.empty((n_nodes, 128), dtype=np.float32)
    for c in range(N_CORES):
        noc = meta["node_of_col"][c]
        valid = noc >= 0
        ot = np.asarray(res.results[c]["outT"]).astype(np.float32)
        out[noc[valid]] = ot[:, valid].T
    return out
